# revision 1
# baseline (speedup 1.0000x reference)
"""Causal self-attention on 8 Trainium2 NeuronCores.

Problem: x[2,2048,2048] f32, W_qkv[2048,6144], W_out[2048,2048]
  qkv = x @ W_qkv; per-head causal softmax attention; out = attn @ W_out.

Sharding: core c handles batch b=c//4, head group hg=c%4 (4 of 16 heads).
Each core computes its heads' QKV projections, full causal attention for
those heads, and a partial output projection (its heads' rows of W_out).
Host sums the 4 partial outputs per batch. x is shipped pre-transposed
(xT[b] = x[b].T) so the device needs no transposes: every matmul wants the
contraction dim (D or Tk or Hd) on partitions.

Device kernel (per core, SPMD, all matmuls fp32r):
  Phase A: qT/kT per head (lhsT=W chunk, rhs=xT slab) and v for all heads
           (lhsT=xT chunk, rhs=Wv block), K=2048 PSUM accumulation.
  Phase B: per head, per 512-wide query group: S^T = kT_blk.T @ qT (keys on
           partitions), E = exp(scale*S^T) (ScalarE -> fp32r), causal mask
           on diagonal blocks via GpSimd affine_select, denominator row via
           ones[128,1] matmul accumulation, attn_outT via v_blk-as-lhsT
           accumulation, normalize by 1/denom broadcast (K=1 ones matmul);
           no max-subtraction (scores ~N(0,1), exp is safe in fp32).
           Normalized attn_outT tiles stream to a DRAM scratch.
  Phase C: y = sum_h attn_outT_h.T @ W_out_h rows, PSUM-accumulated over
           the 4 local heads; attn_outT tiles reloaded from scratch.
"""
import math

import numpy as np

import concourse.bass as bass
import concourse.mybir as mybir
import concourse.tile as tile
from concourse import bacc
from concourse.bass_utils import run_bass_kernel_spmd

B, T, D = 2, 2048, 2048
H, Hd = 16, 128
N_CORES = 8
HL = 4            # heads per core
DL = HL * Hd      # 512: local hidden slice
P = 128
KC = D // P       # 16 contraction chunks of 128
NTB = T // P      # 16 row blocks of 128
QTW = 512         # query-group width
NQT = T // QTW    # 4 query groups
SCALE = 1.0 / math.sqrt(Hd)

f32 = mybir.dt.float32
f32r = mybir.dt.float32r
AF = mybir.ActivationFunctionType


def build_program(reps: int = 1, phases: str = "ABC"):
    nc = bacc.Bacc("TRN2", target_bir_lowering=False, debug=False,
                   num_devices=N_CORES)
    xT = nc.dram_tensor("xT", [D, T], f32r, kind="ExternalInput")
    wq = nc.dram_tensor("wq", [D, DL], f32r, kind="ExternalInput")
    wk = nc.dram_tensor("wk", [D, DL], f32r, kind="ExternalInput")
    wv = nc.dram_tensor("wv", [D, DL], f32r, kind="ExternalInput")
    wout = nc.dram_tensor("wout", [DL, D], f32r, kind="ExternalInput")
    y = nc.dram_tensor("y", [T, D], f32, kind="ExternalOutput")

    with tile.TileContext(nc) as tc:
        if reps > 1:
            with tc.For_i(0, reps, 1):
                _body(nc, tc, xT, wq, wk, wv, wout, y, phases)
        else:
            _body(nc, tc, xT, wq, wk, wv, wout, y, phases)
    nc.compile()
    return nc


def _body(nc, tc, xT, wq, wk, wv, wout, y, phases="ABC"):
    with (
        tc.tile_pool(name="persist", bufs=1) as persist,
        tc.tile_pool(name="dram", bufs=1, space="DRAM") as dram,
    ):
        # attn_outT scratch: [h, Hd, T] fp32r
        atT_dram = dram.tile([HL, Hd, T], f32r)

        # f32r constants (built from f32 scratch, rounded via tensor_copy)
        ones_col = persist.tile([P, 1], f32r)      # lhsT for denom matmul
        ones_row = persist.tile([1, P], f32r)      # lhsT for K=1 broadcast
        # shifted-triangular causal mask base: base[i, c] = 1.0 iff c >= i+384.
        # diagonal block m (key-block kb = 4*qt+m) uses slice
        # base[:, (3-m)*128 : (3-m)*128+512]: keep iff j >= i + m*128.
        maskb = persist.tile([P, 896], f32r)
        with tc.tile_pool(name="init_scratch", bufs=1) as scratch:
            ones_f = scratch.tile([P, 1], f32)
            nc.vector.memset(ones_f[:], 1.0)
            nc.vector.tensor_copy(ones_col[:], ones_f[:])
            ones1_f = scratch.tile([1, P], f32)
            nc.vector.memset(ones1_f[:], 1.0)
            nc.vector.tensor_copy(ones_row[:], ones1_f[:])
            mask_f = scratch.tile([P, 896], f32)
            nc.gpsimd.memset(mask_f[:], 1.0)
            nc.gpsimd.affine_select(
                out=mask_f[:], in_=mask_f[:],
                compare_op=mybir.AluOpType.is_ge,
                fill=0.0, base=-384, channel_multiplier=-1,
                pattern=[[1, 896]])
            nc.vector.tensor_copy(maskb[:], mask_f[:])

        with tc.tile_pool(name="qkv", bufs=1) as qkv_pool:
            qT_sb = qkv_pool.tile([P, HL, T], f32r)   # [Hd, h, Tq]
            kT_sb = qkv_pool.tile([P, HL, T], f32r)
            v_sb = qkv_pool.tile([P, NTB, DL], f32r)  # [Tk%128, kb, h*Hd]

            # ------------ Phase A: QKV projection ------------------------
            with (
                tc.tile_pool(name="a_xT", bufs=2) as xTpool,
                tc.tile_pool(name="a_w", bufs=2) as wpool,
                tc.tile_pool(name="a_wv", bufs=3) as wvpool,
                tc.tile_pool(name="ps_qk", bufs=3, space="PSUM") as ps_qk,
                tc.tile_pool(name="ps_v", bufs=4, space="PSUM") as ps_v,
            ):
                for s in range(NQT):  # 4 slabs of 512 T-cols
                    xTs = xTpool.tile([P, KC, QTW], f32r, tag="xT")
                    nc.sync.dma_start(
                        xTs[:],
                        xT.ap()[:, s * QTW:(s + 1) * QTW].rearrange(
                            "(kc p) t -> p kc t", p=P))
                    # qT / kT for the 4 local heads
                    for h in range(HL):
                        for wdram, dst in ((wq, qT_sb), (wk, kT_sb)):
                            wt = wpool.tile([P, KC, Hd], f32r, tag="w_qk")
                            nc.sync.dma_start(
                                wt[:],
                                wdram.ap()[:, h * Hd:(h + 1) * Hd].rearrange(
                                    "(kc p) m -> p kc m", p=P))
                            ps = ps_qk.tile([P, QTW], f32, tag="qk",
                                            name="qk_ps")
                            for kc in range(KC):
                                nc.tensor.matmul(
                                    ps[:], wt[:, kc, :], xTs[:, kc, :],
                                    start=(kc == 0), stop=(kc == KC - 1))
                            nc.vector.tensor_copy(
                                dst[:, h, s * QTW:(s + 1) * QTW], ps[:])
                    # v for all 4 heads (kc-outer so wv streams once per slab)
                    vps = [ps_v.tile([P, DL], f32, tag="v", name=f"vps{i}")
                           for i in range(4)]
                    for kc in range(KC):
                        wvt = wvpool.tile([P, DL], f32r, tag="wv")
                        nc.sync.dma_start(
                            wvt[:], wv.ap()[kc * P:(kc + 1) * P, :])
                        for tsub in range(4):
                            nc.tensor.matmul(
                                vps[tsub][:],
                                xTs[:, kc, tsub * P:(tsub + 1) * P],
                                wvt[:],
                                start=(kc == 0), stop=(kc == KC - 1))
                    for tsub in range(4):
                        nc.vector.tensor_copy(
                            v_sb[:, s * 4 + tsub, :], vps[tsub][:])

            # ------ Phases B+C fused: attention interleaved with out-proj ----
            # qt-outer: after query-group qt finishes for all heads, the
            # 4 output row-blocks of that group run their out-projection,
            # filling TensorE gaps left by the exp-latency chain.
            if "B" not in phases:
                return
            do_c = "C" in phases
            with (
                tc.tile_pool(name="b_e", bufs=5) as epool,
                tc.tile_pool(name="b_small", bufs=2) as bsmall,
                tc.tile_pool(name="b_out", bufs=2) as boutpool,
                tc.tile_pool(name="c_w", bufs=1) as cwpool,
                tc.tile_pool(name="c_at", bufs=3) as atpool,
                tc.tile_pool(name="c_y", bufs=2) as ypool,
                tc.tile_pool(name="ps_s", bufs=3, space="PSUM") as ps_s,
                tc.tile_pool(name="ps_d", bufs=1, space="PSUM") as ps_d,
                tc.tile_pool(name="ps_o", bufs=2, space="PSUM") as ps_o,
                tc.tile_pool(name="ps_y", bufs=2, space="PSUM") as ps_y,
            ):
                if do_c:
                    wout_sb = cwpool.tile([P, HL, D], f32r)
                    nc.sync.dma_start(
                        wout_sb[:],
                        wout.ap().rearrange("(hl p) d -> p hl d", p=P))
                for qt in range(NQT):
                    for h in range(HL):
                        nkb = (qt + 1) * 4
                        q_sl = slice(qt * QTW, (qt + 1) * QTW)
                        d_ps = ps_d.tile([1, QTW], f32, tag="d", name="d_ps")
                        o_ps = ps_o.tile([P, QTW], f32, tag="o", name="o_ps")
                        for kb in range(nkb):
                            s_ps = ps_s.tile([P, QTW], f32, tag="s",
                                             name="s_ps")
                            nc.tensor.matmul(
                                s_ps[:],
                                kT_sb[:, h, kb * P:(kb + 1) * P],
                                qT_sb[:, h, q_sl],
                                start=True, stop=True)
                            e_sb = epool.tile([P, QTW], f32r, tag="e")
                            nc.scalar.activation(
                                e_sb[:], s_ps[:], AF.Exp, scale=float(SCALE))
                            if kb >= 4 * qt:
                                # diagonal block: keep q >= k, i.e.
                                # j >= i + m*128 with m = kb - 4*qt
                                m = kb - 4 * qt
                                off = (3 - m) * P
                                nc.vector.tensor_mul(
                                    e_sb[:], e_sb[:],
                                    maskb[:, off:off + QTW])
                            nc.tensor.matmul(
                                d_ps[:], ones_col[:], e_sb[:],
                                start=(kb == 0), stop=(kb == nkb - 1))
                            nc.tensor.matmul(
                                o_ps[:], v_sb[:, kb, h * Hd:(h + 1) * Hd],
                                e_sb[:],
                                start=(kb == 0), stop=(kb == nkb - 1))
                        # copy the AV accumulator out early to release its
                        # PSUM bank before the (serial) normalize tail
                        o_raw = bsmall.tile([P, QTW], f32, tag="oraw")
                        nc.vector.tensor_copy(o_raw[:], o_ps[:])
                        rec = bsmall.tile([1, QTW], f32r, tag="rec")
                        with nc.allow_low_precision(
                                reason="f32r reciprocal, 2^-19 rel"):
                            nc.vector.reciprocal(rec[:], d_ps[:])
                        bc_ps = ps_y.tile([P, QTW], f32, tag="y",
                                          name="bc_ps")
                        nc.tensor.matmul(
                            bc_ps[:], ones_row[:], rec[:],
                            start=True, stop=True)
                        at_sb = boutpool.tile([P, QTW], f32r, tag="at")
                        nc.vector.tensor_mul(at_sb[:], o_raw[:], bc_ps[:])
                        nc.sync.dma_start(atT_dram[h, :, q_sl], at_sb[:])
                    if not do_c:
                        continue
                    # out-projection for this query group's 4 row blocks
                    for tb in range(qt * 4, qt * 4 + 4):
                        t_sl = slice(tb * P, (tb + 1) * P)
                        ats = atpool.tile([P, HL, P], f32r, tag="at_in")
                        for h in range(HL):
                            nc.sync.dma_start(
                                ats[:, h, :], atT_dram[h, :, t_sl])
                        y_sb = ypool.tile([P, D], f32, tag="ysb")
                        for dc in range(D // QTW):
                            y_ps = ps_y.tile([P, QTW], f32, tag="y",
                                             name="y_ps")
                            for h in range(HL):
                                nc.tensor.matmul(
                                    y_ps[:],
                                    ats[:, h, :],
                                    wout_sb[:, h, dc * QTW:(dc + 1) * QTW],
                                    start=(h == 0), stop=(h == HL - 1))
                            nc.vector.tensor_copy(
                                y_sb[:, dc * QTW:(dc + 1) * QTW], y_ps[:])
                        nc.sync.dma_start(y.ap()[t_sl, :], y_sb[:])


def prepare_in_maps(x, W_qkv, W_out):
    x = np.ascontiguousarray(np.asarray(x), dtype=np.float32)
    W_qkv = np.ascontiguousarray(np.asarray(W_qkv), dtype=np.float32)
    W_out = np.ascontiguousarray(np.asarray(W_out), dtype=np.float32)
    Wr = W_qkv.reshape(D, 3, H, Hd)
    Wo = W_out.reshape(H, Hd, D)
    xTs = [np.ascontiguousarray(x[b].T) for b in range(B)]
    in_maps = []
    for c in range(N_CORES):
        b, hg = c // 4, c % 4
        hs = slice(hg * HL, (hg + 1) * HL)
        in_maps.append({
            "xT": xTs[b],
            "wq": np.ascontiguousarray(Wr[:, 0, hs, :].reshape(D, DL)),
            "wk": np.ascontiguousarray(Wr[:, 1, hs, :].reshape(D, DL)),
            "wv": np.ascontiguousarray(Wr[:, 2, hs, :].reshape(D, DL)),
            "wout": np.ascontiguousarray(Wo[hs].reshape(DL, D)),
        })
    return in_maps


def combine_outputs(results):
    out = np.zeros((B, T, D), dtype=np.float32)
    for c in range(N_CORES):
        out[c // 4] += results[c]["y"]
    return out


_PROGRAM_CACHE = {}


def kernel(x, W_qkv, W_out):
    in_maps = prepare_in_maps(x, W_qkv, W_out)
    if 1 not in _PROGRAM_CACHE:
        _PROGRAM_CACHE[1] = build_program(1)
    nc = _PROGRAM_CACHE[1]
    res = run_bass_kernel_spmd(nc, in_maps, core_ids=list(range(N_CORES)))
    return combine_outputs(res.results)



# revision 2
# speedup vs baseline: 1.1727x; 1.1727x over previous
"""Causal self-attention on 8 Trainium2 NeuronCores.

Problem: x[2,2048,2048] f32, W_qkv[2048,6144], W_out[2048,2048]
  qkv = x @ W_qkv; per-head causal softmax attention; out = attn @ W_out.

Sharding: core c handles batch b=c//4, head group hg=c%4 (4 of 16 heads).
Each core computes its heads' QKV projections, full causal attention for
those heads, and a partial output projection (its heads' rows of W_out).
Host sums the 4 partial outputs per batch.

All device matmuls run in bf16 (fp32 PSUM accumulation): same PE rate as
fp32r at 512-wide outputs, but enables FWL fast weight loads and halves
DMA + SBUF. Host pre-packs every tensor so each DMA is a large
contiguous-per-partition transfer, and all weights are loaded into SBUF
exactly once.

Per-slab interleave keeps the PE warm end to end: for each 512-query
slab s: [A] project q/k for the 4 local heads + v (weights resident,
xT slab double-buffered), then per head h: [B] causal attention for
query group s (scores -> exp -> AV, with diagonal blocks narrowed to
the unmasked query range; softmax denominator accumulated on VectorE,
reduced+broadcast via one GpSimd partition_all_reduce, so TensorE does
no reduction work), then [C] one 128-row block of the output projection
for query group s-1 (attention outputs live in SBUF; the one-group lag
guarantees their normalize tails are long done, and the C matmuls fill
TensorE bubbles left by the exp-latency chain).
"""
import math

import ml_dtypes
import numpy as np

import concourse.bass as bass
import concourse.bass_isa as bass_isa
import concourse.mybir as mybir
import concourse.tile as tile
from concourse import bacc
from concourse.bass_utils import run_bass_kernel_spmd

B, T, D = 2, 2048, 2048
H, Hd = 16, 128
N_CORES = 8
HL = 4            # heads per core
DL = HL * Hd      # 512: local hidden slice
P = 128
KC = D // P       # 16 contraction chunks of 128
NTB = T // P      # 16 row blocks of 128
QTW = 512         # query-group width
NQT = T // QTW    # 4 query groups
SCALE = 1.0 / math.sqrt(Hd)

f32 = mybir.dt.float32
bf16 = mybir.dt.bfloat16
AF = mybir.ActivationFunctionType


def build_program(reps: int = 1, phases: str = "ABC"):
    nc = bacc.Bacc("TRN2", target_bir_lowering=False, debug=False,
                   num_devices=N_CORES)
    # host-packed layouts (partition dim first, contiguous per partition)
    xT = nc.dram_tensor("xT", [P, NQT, KC, QTW], bf16, kind="ExternalInput")
    wq = nc.dram_tensor("wq", [P, KC, DL], bf16, kind="ExternalInput")
    wk = nc.dram_tensor("wk", [P, KC, DL], bf16, kind="ExternalInput")
    wv = nc.dram_tensor("wv", [P, KC, DL], bf16, kind="ExternalInput")
    wout = nc.dram_tensor("wout", [P, HL, D], bf16, kind="ExternalInput")
    y = nc.dram_tensor("y", [T, D], f32, kind="ExternalOutput")

    with tile.TileContext(nc) as tc:
        if reps > 1:
            with tc.For_i(0, reps, 1):
                _body(nc, tc, xT, wq, wk, wv, wout, y, phases)
        else:
            _body(nc, tc, xT, wq, wk, wv, wout, y, phases)
    nc.compile()
    return nc


def _body(nc, tc, xT, wq, wk, wv, wout, y, phases="ABC"):
    do_b = "B" in phases
    do_c = "C" in phases
    with (
        tc.tile_pool(name="weights", bufs=1) as wpool,
        tc.tile_pool(name="qkv", bufs=1) as qkv_pool,
        tc.tile_pool(name="xts", bufs=2) as xpool,
        tc.tile_pool(name="e", bufs=6) as epool,
        tc.tile_pool(name="eacc", bufs=2) as eaccpool,
        tc.tile_pool(name="dbc", bufs=2) as dpool,
        tc.tile_pool(name="rbc", bufs=2) as rpool,
        tc.tile_pool(name="ysb", bufs=2) as ypool,
        tc.tile_pool(name="ps_mm", bufs=2, space="PSUM") as pmm,
        tc.tile_pool(name="ps_s", bufs=4, space="PSUM") as ps_s,
        tc.tile_pool(name="ps_o", bufs=2, space="PSUM") as po,
    ):
        wq_sb = wpool.tile([P, KC, DL], bf16)
        wk_sb = wpool.tile([P, KC, DL], bf16)
        wv_sb = wpool.tile([P, KC, DL], bf16)
        wout_sb = wpool.tile([P, HL, D], bf16)
        qT_sb = qkv_pool.tile([P, HL, T], bf16)   # [Hd, h, Tq]
        kT_sb = qkv_pool.tile([P, HL, T], bf16)
        v_sb = qkv_pool.tile([P, NTB, DL], bf16)  # [Tk%128, kb, h*Hd]
        atT_sb = qkv_pool.tile([P, HL, T], bf16)  # normalized attn outT

        nc.sync.dma_start(wq_sb[:], wq.ap())
        nc.sync.dma_start(wk_sb[:], wk.ap())
        nc.sync.dma_start(wv_sb[:], wv.ap())
        if do_c:
            nc.sync.dma_start(wout_sb[:], wout.ap())

        def b_head(qt, h):
            o_ps = po.tile([P, QTW], f32, tag="o", name="o_ps")
            e_acc = eaccpool.tile([P, QTW], f32, tag="eacc")
            nkb = (qt + 1) * 4
            for kb in range(nkb):
                m = kb - 4 * qt  # >=0: diagonal block, narrow to live cols
                off = max(m, 0) * P
                q_sl = slice(qt * QTW + off, (qt + 1) * QTW)
                s_ps = ps_s.tile([P, QTW], f32, tag="s", name="s_ps")
                nc.tensor.matmul(
                    s_ps[:, off:],
                    kT_sb[:, h, kb * P:(kb + 1) * P],
                    qT_sb[:, h, q_sl],
                    start=True, stop=True)
                e_sb = epool.tile([P, QTW], bf16, tag="e")
                with nc.allow_low_precision(reason="bf16 attn weights"):
                    nc.scalar.activation(
                        e_sb[:, off:], s_ps[:, off:], AF.Exp,
                        scale=float(SCALE))
                if m >= 0:
                    # keep q >= k: within the narrowed block this is
                    # col_local >= channel
                    nc.gpsimd.affine_select(
                        out=e_sb[:, off:], in_=e_sb[:, off:],
                        compare_op=mybir.AluOpType.is_ge,
                        fill=0.0, base=0, channel_multiplier=-1,
                        pattern=[[1, QTW - off]])
                nc.tensor.matmul(
                    o_ps[:, off:],
                    v_sb[:, kb, h * Hd:(h + 1) * Hd],
                    e_sb[:, off:],
                    start=(kb == 0), stop=(kb == nkb - 1))
                if kb == 0:
                    nc.vector.tensor_copy(e_acc[:], e_sb[:])
                else:
                    nc.vector.tensor_add(
                        e_acc[:, off:], e_acc[:, off:], e_sb[:, off:])
            # softmax denominator: partition-sum broadcast to all rows,
            # then normalize straight into the persistent attn-out tile
            d_bc = dpool.tile([P, QTW], f32, tag="dbc")
            nc.gpsimd.partition_all_reduce(
                d_bc[:], e_acc[:], channels=P, reduce_op=bass_isa.ReduceOp.add)
            r_bc = rpool.tile([P, QTW], bf16, tag="rbc")
            with nc.allow_low_precision(reason="bf16 softmax recip"):
                nc.vector.reciprocal(r_bc[:], d_bc[:])
                nc.vector.tensor_mul(
                    atT_sb[:, h, qt * QTW:(qt + 1) * QTW], o_ps[:], r_bc[:])

        def c_tb(tb):
            t_sl = slice(tb * P, (tb + 1) * P)
            y_sb = ypool.tile([P, D], f32, tag="ysb")
            for dc in range(D // QTW):
                y_ps = pmm.tile([P, QTW], f32, tag="mm", name="y_ps")
                for h in range(HL):
                    nc.tensor.matmul(
                        y_ps[:],
                        atT_sb[:, h, t_sl],
                        wout_sb[:, h, dc * QTW:(dc + 1) * QTW],
                        start=(h == 0), stop=(h == HL - 1))
                nc.vector.tensor_copy(y_sb[:, dc * QTW:(dc + 1) * QTW],
                                      y_ps[:])
            nc.sync.dma_start(y.ap()[t_sl, :], y_sb[:])

        for s in range(NQT):
            # ---- A: qkv projection for this 512-query slab --------------
            xTs = xpool.tile([P, KC, QTW], bf16, tag="xT")
            nc.sync.dma_start(xTs[:], xT.ap()[:, s])
            for h in range(HL):
                for w_sb, dst in ((wq_sb, qT_sb), (wk_sb, kT_sb)):
                    ps = pmm.tile([P, QTW], f32, tag="mm", name="qk_ps")
                    for kc in range(KC):
                        nc.tensor.matmul(
                            ps[:], w_sb[:, kc, h * Hd:(h + 1) * Hd],
                            xTs[:, kc, :],
                            start=(kc == 0), stop=(kc == KC - 1))
                    with nc.allow_low_precision(reason="bf16 qkv"):
                        nc.vector.tensor_copy(
                            dst[:, h, s * QTW:(s + 1) * QTW], ps[:])
            for tsub in range(4):
                ps = pmm.tile([P, DL], f32, tag="mm", name="v_ps")
                for kc in range(KC):
                    nc.tensor.matmul(
                        ps[:], xTs[:, kc, tsub * P:(tsub + 1) * P],
                        wv_sb[:, kc, :],
                        start=(kc == 0), stop=(kc == KC - 1))
                with nc.allow_low_precision(reason="bf16 qkv"):
                    nc.vector.tensor_copy(v_sb[:, s * 4 + tsub, :], ps[:])
            # ---- B qt=s, interleaved with C for query group s-1 ---------
            if not do_b:
                continue
            for h in range(HL):
                b_head(s, h)
                if do_c and s > 0:
                    c_tb(4 * (s - 1) + h)
        if do_b and do_c:
            for tb in range(4 * (NQT - 1), 4 * NQT):
                c_tb(tb)


def prepare_in_maps(x, W_qkv, W_out):
    x = np.ascontiguousarray(np.asarray(x), dtype=np.float32)
    W_qkv = np.ascontiguousarray(np.asarray(W_qkv), dtype=np.float32)
    W_out = np.ascontiguousarray(np.asarray(W_out), dtype=np.float32)
    bf = ml_dtypes.bfloat16
    Wr = W_qkv.reshape(D, 3, H, Hd)
    Wo = W_out.reshape(H, Hd, D)
    # xT[b]: [D,T] -> [p, s, kc, t]
    xTs = [np.ascontiguousarray(
        x[b].T.reshape(KC, P, NQT, QTW).transpose(1, 2, 0, 3).astype(bf))
        for b in range(B)]

    def packw(w):  # [D, DL] -> [p, kc, DL]
        return np.ascontiguousarray(
            w.reshape(KC, P, DL).transpose(1, 0, 2).astype(bf))

    in_maps = []
    for c in range(N_CORES):
        b, hg = c // 4, c % 4
        hs = slice(hg * HL, (hg + 1) * HL)
        in_maps.append({
            "xT": xTs[b],
            "wq": packw(Wr[:, 0, hs, :].reshape(D, DL)),
            "wk": packw(Wr[:, 1, hs, :].reshape(D, DL)),
            "wv": packw(Wr[:, 2, hs, :].reshape(D, DL)),
            "wout": np.ascontiguousarray(
                Wo[hs].transpose(1, 0, 2).astype(bf)),
        })
    return in_maps


def combine_outputs(results):
    out = np.zeros((B, T, D), dtype=np.float32)
    for c in range(N_CORES):
        out[c // 4] += results[c]["y"]
    return out


_PROGRAM_CACHE = {}


def kernel(x, W_qkv, W_out):
    in_maps = prepare_in_maps(x, W_qkv, W_out)
    if 1 not in _PROGRAM_CACHE:
        _PROGRAM_CACHE[1] = build_program(1)
    nc = _PROGRAM_CACHE[1]
    res = run_bass_kernel_spmd(nc, in_maps, core_ids=list(range(N_CORES)))
    return combine_outputs(res.results)


# revision 8
# speedup vs baseline: 1.3848x; 1.1809x over previous
"""Causal self-attention on 8 Trainium2 NeuronCores.

Problem: x[2,2048,2048] f32, W_qkv[2048,6144], W_out[2048,2048]
  qkv = x @ W_qkv; per-head causal softmax attention; out = attn @ W_out.

Sharding: core c handles batch b=c//4, head group hg=c%4 (4 of 16 heads).
Each core computes its heads' QKV projections, full causal attention for
those heads, and a partial output projection (its heads' rows of W_out).
Host sums the 4 partial outputs per batch.

All device matmuls run in bf16 (fp32 PSUM accumulation): same PE rate as
fp32r at 512-wide outputs, but enables FWL fast weight loads and halves
DMA + SBUF. Host pre-packs every tensor so each DMA is a large
contiguous-per-partition transfer, and all weights are loaded into SBUF
exactly once (per-head q/k chunks so the first matmul group starts after
~2.5MB of DMA).

Per-slab interleave keeps the PE warm end to end: for each 512-query
slab s: [A] project q/k for the 4 local heads + v (weights resident,
xT slab double-buffered), then per head h: [B] causal attention for
query group s, then [C] one 128-row block of the output projection for
query group s-1 (attention outputs live in SBUF; the one-group lag
guarantees their normalize tails are done, and the C matmuls fill
TensorE bubbles left by the exp-latency chain).

Attention per (group, head): S^T = k_blk^T.T @ q^T with keys on
partitions; diagonal key-blocks are narrowed to their live query range
so S/exp/AV/denominator all skip fully-masked work, and only the first
128 columns of each diagonal block (the triangular part) get a
precomputed bf16 causal-mask multiply on VectorE. The softmax
denominator accumulates on TensorE as a [1,512] ones-matmul per
key-block (cheap: cost scales with streamed rows only), the reciprocal
uses the fast custom-DVE approximation, and a K=1 f32r ones-matmul
broadcasts it across partitions for the normalize multiply.
"""
import math

import ml_dtypes
import numpy as np

import concourse.bass as bass
import concourse.mybir as mybir
import concourse.tile as tile
from concourse import bacc
from concourse.bass_utils import run_bass_kernel_spmd

B, T, D = 2, 2048, 2048
H, Hd = 16, 128
N_CORES = 8
HL = 4            # heads per core
DL = HL * Hd      # 512: local hidden slice
P = 128
KC = D // P       # 16 contraction chunks of 128
NTB = T // P      # 16 row blocks of 128
QTW = 512         # query-group width
NQT = T // QTW    # 4 query groups
SCALE = 1.0 / math.sqrt(Hd)

f32 = mybir.dt.float32
f32r = mybir.dt.float32r
bf16 = mybir.dt.bfloat16
AF = mybir.ActivationFunctionType


def build_program(reps: int = 1, phases: str = "ABC"):
    nc = bacc.Bacc("TRN2", target_bir_lowering=False, debug=False,
                   num_devices=N_CORES)
    # host-packed layouts (partition dim first, contiguous per partition)
    xT = nc.dram_tensor("xT", [P, NQT, KC, QTW], bf16, kind="ExternalInput")
    wq = nc.dram_tensor("wq", [P, HL, KC, Hd], bf16, kind="ExternalInput")
    wk = nc.dram_tensor("wk", [P, HL, KC, Hd], bf16, kind="ExternalInput")
    wv = nc.dram_tensor("wv", [P, KC, DL], bf16, kind="ExternalInput")
    wout = nc.dram_tensor("wout", [P, HL, D], bf16, kind="ExternalInput")
    y = nc.dram_tensor("y", [T, D], bf16, kind="ExternalOutput")

    with tile.TileContext(nc) as tc:
        if reps > 1:
            with tc.For_i(0, reps, 1):
                _body(nc, tc, xT, wq, wk, wv, wout, y, phases)
        else:
            _body(nc, tc, xT, wq, wk, wv, wout, y, phases)
    nc.compile()
    return nc


def _body(nc, tc, xT, wq, wk, wv, wout, y, phases="ABC"):
    do_b = "B" in phases
    do_c = "C" in phases
    with (
        tc.tile_pool(name="weights", bufs=1) as wpool,
        tc.tile_pool(name="qkv", bufs=1) as qkv_pool,
        tc.tile_pool(name="consts", bufs=1) as cpool,
        tc.tile_pool(name="xts", bufs=2) as xpool,
        tc.tile_pool(name="e", bufs=6) as epool,
        tc.tile_pool(name="rec", bufs=2) as recpool,
        tc.tile_pool(name="oraw", bufs=2) as opool,
        tc.tile_pool(name="ysb", bufs=2) as ypool,
        tc.tile_pool(name="psum", bufs=1, space="PSUM") as psum,
    ):
        # causal mask / ones constants (f32 scratch -> target dtypes)
        ones_col = cpool.tile([P, 1], bf16)     # lhsT for denom matmul
        ones_row = cpool.tile([1, P], bf16)     # lhsT for K=1 broadcast
        tri = cpool.tile([P, P], bf16)          # tri[r, j] = 1.0 iff j >= r
        with tc.tile_pool(name="init_scratch", bufs=1) as scratch:
            sc = scratch.tile([P, P], f32)
            nc.gpsimd.memset(sc[:], 1.0)
            with nc.allow_low_precision(reason="0/1 mask exact in bf16"):
                nc.vector.tensor_copy(ones_col[:], sc[:, :1])
                nc.vector.tensor_copy(ones_row[:], sc[:1, :])
            nc.gpsimd.affine_select(
                out=sc[:], in_=sc[:],
                compare_op=mybir.AluOpType.is_ge,
                fill=0.0, base=0, channel_multiplier=-1,
                pattern=[[1, P]])
            with nc.allow_low_precision(reason="0/1 mask exact in bf16"):
                nc.vector.tensor_copy(tri[:], sc[:])

        wq_sb = wpool.tile([P, HL, KC, Hd], bf16)
        wk_sb = wpool.tile([P, HL, KC, Hd], bf16)
        wv_sb = wpool.tile([P, KC, DL], bf16)
        wout_sb = wpool.tile([P, HL, D], bf16)
        qT_sb = qkv_pool.tile([P, HL, T], bf16)   # [Hd, h, Tq]
        kT_sb = qkv_pool.tile([P, HL, T], bf16)
        v_sb = qkv_pool.tile([P, NTB, DL], bf16)  # [Tk%128, kb, h*Hd]
        atT_sb = qkv_pool.tile([P, HL, T], bf16)  # normalized attn outT

        def b_head(qt, h):
            o_ps = psum.tile([P, QTW], f32, tag="o", bufs=2, name="o_ps")
            d_ps = psum.tile([1, QTW], f32, tag="d", bufs=1, name="d_ps")
            nkb = (qt + 1) * 4
            for kb in range(nkb):
                m = kb - 4 * qt  # >=0: diagonal block, narrow to live cols
                off = max(m, 0) * P
                q_sl = slice(qt * QTW + off, (qt + 1) * QTW)
                s_ps = psum.tile([P, QTW], f32, tag="s", bufs=3, name="s_ps")
                nc.tensor.matmul(
                    s_ps[:, off:],
                    kT_sb[:, h, kb * P:(kb + 1) * P],
                    qT_sb[:, h, q_sl],
                    start=True, stop=True)
                e_sb = epool.tile([P, QTW], bf16, tag="e")
                with nc.allow_low_precision(reason="bf16 attn weights"):
                    nc.scalar.activation(
                        e_sb[:, off:], s_ps[:, off:], AF.Exp,
                        scale=float(SCALE))
                if m >= 0:
                    # only the first 128 live columns are partially masked
                    nc.vector.tensor_mul(
                        e_sb[:, off:off + P], e_sb[:, off:off + P], tri[:])
                nc.tensor.matmul(
                    o_ps[:, off:],
                    v_sb[:, kb, h * Hd:(h + 1) * Hd],
                    e_sb[:, off:],
                    start=(kb == 0), stop=(kb == nkb - 1))
                nc.tensor.matmul(
                    d_ps[:, off:], ones_col[:], e_sb[:, off:],
                    start=(kb == 0), stop=(kb == nkb - 1))
            rec = recpool.tile([1, QTW], f32, tag="rec")
            nc.vector.reciprocal_approx_fast(rec[:], d_ps[:])
            rec_bf = recpool.tile([1, QTW], bf16, tag="recbf")
            with nc.allow_low_precision(reason="bf16 softmax recip"):
                nc.vector.tensor_copy(rec_bf[:], rec[:])
            bc_ps = psum.tile([P, QTW], f32, tag="s", bufs=3, name="bc_ps")
            nc.tensor.matmul(
                bc_ps[:], ones_row[:], rec_bf[:],
                start=True, stop=True)
            o_raw = opool.tile([P, QTW], bf16, tag="oraw")
            with nc.allow_low_precision(reason="bf16 attn out"):
                nc.vector.tensor_copy(o_raw[:], o_ps[:])
                nc.vector.tensor_mul(
                    atT_sb[:, h, qt * QTW:(qt + 1) * QTW], o_raw[:],
                    bc_ps[:])

        def c_tb(tb):
            t_sl = slice(tb * P, (tb + 1) * P)
            y_sb = ypool.tile([P, D], f32, tag="ysb")
            for dc in range(D // QTW):
                y_ps = psum.tile([P, QTW], f32, tag="mm", bufs=2, name="y_ps")
                for h in range(HL):
                    nc.tensor.matmul(
                        y_ps[:],
                        atT_sb[:, h, t_sl],
                        wout_sb[:, h, dc * QTW:(dc + 1) * QTW],
                        start=(h == 0), stop=(h == HL - 1))
                nc.vector.tensor_copy(y_sb[:, dc * QTW:(dc + 1) * QTW],
                                      y_ps[:])
            nc.sync.dma_start(y.ap()[t_sl, :], y_sb[:])

        # first xT slab first so the first matmul group isn't queued
        # behind the full weight load; wout (not needed until the first
        # C block, one slab later) goes last
        xTs0 = xpool.tile([P, KC, QTW], bf16, tag="xT")
        nc.sync.dma_start(xTs0[:], xT.ap()[:, 0])
        for h in range(HL):
            nc.sync.dma_start(wq_sb[:, h], wq.ap()[:, h])
            nc.sync.dma_start(wk_sb[:, h], wk.ap()[:, h])
        nc.sync.dma_start(wv_sb[:], wv.ap())
        if do_c:
            nc.sync.dma_start(wout_sb[:], wout.ap())

        for s in range(NQT):
            # ---- A: qkv projection for this 512-query slab --------------
            if s == 0:
                xTs = xTs0
            else:
                xTs = xpool.tile([P, KC, QTW], bf16, tag="xT")
                nc.sync.dma_start(xTs[:], xT.ap()[:, s])
            for h in range(HL):
                for w_sb, dst in ((wq_sb, qT_sb), (wk_sb, kT_sb)):
                    ps = psum.tile([P, QTW], f32, tag="mm", bufs=2,
                                   name="qk_ps")
                    for kc in range(KC):
                        nc.tensor.matmul(
                            ps[:], w_sb[:, h, kc], xTs[:, kc, :],
                            start=(kc == 0), stop=(kc == KC - 1))
                    with nc.allow_low_precision(reason="bf16 qkv"):
                        nc.vector.tensor_copy(
                            dst[:, h, s * QTW:(s + 1) * QTW], ps[:])
            for tsub in range(4):
                ps = psum.tile([P, DL], f32, tag="mm", bufs=2, name="v_ps")
                for kc in range(KC):
                    nc.tensor.matmul(
                        ps[:], xTs[:, kc, tsub * P:(tsub + 1) * P],
                        wv_sb[:, kc, :],
                        start=(kc == 0), stop=(kc == KC - 1))
                with nc.allow_low_precision(reason="bf16 qkv"):
                    nc.vector.tensor_copy(v_sb[:, s * 4 + tsub, :], ps[:])
            # ---- B qt=s, interleaved with C for query group s-1 ---------
            if not do_b:
                continue
            for h in range(HL):
                b_head(s, h)
                if do_c and s > 0:
                    c_tb(4 * (s - 1) + h)
        if do_b and do_c:
            for tb in range(4 * (NQT - 1), 4 * NQT):
                c_tb(tb)


def prepare_in_maps(x, W_qkv, W_out):
    x = np.ascontiguousarray(np.asarray(x), dtype=np.float32)
    W_qkv = np.ascontiguousarray(np.asarray(W_qkv), dtype=np.float32)
    W_out = np.ascontiguousarray(np.asarray(W_out), dtype=np.float32)
    bf = ml_dtypes.bfloat16
    Wr = W_qkv.reshape(D, 3, H, Hd)
    Wo = W_out.reshape(H, Hd, D)
    # xT[b]: [D,T] -> [p, s, kc, t]
    xTs = [np.ascontiguousarray(
        x[b].T.reshape(KC, P, NQT, QTW).transpose(1, 2, 0, 3).astype(bf))
        for b in range(B)]

    def packw_h(w):  # [D, DL] -> [p, h, kc, hd]
        return np.ascontiguousarray(
            w.reshape(KC, P, HL, Hd).transpose(1, 2, 0, 3).astype(bf))

    def packw(w):  # [D, DL] -> [p, kc, DL]
        return np.ascontiguousarray(
            w.reshape(KC, P, DL).transpose(1, 0, 2).astype(bf))

    in_maps = []
    for c in range(N_CORES):
        b, hg = c // 4, c % 4
        hs = slice(hg * HL, (hg + 1) * HL)
        in_maps.append({
            "xT": xTs[b],
            "wq": packw_h(Wr[:, 0, hs, :].reshape(D, DL)),
            "wk": packw_h(Wr[:, 1, hs, :].reshape(D, DL)),
            "wv": packw(Wr[:, 2, hs, :].reshape(D, DL)),
            "wout": np.ascontiguousarray(
                Wo[hs].transpose(1, 0, 2).astype(bf)),
        })
    return in_maps


def combine_outputs(results):
    out = np.zeros((B, T, D), dtype=np.float32)
    for c in range(N_CORES):
        out[c // 4] += results[c]["y"]
    return out


_PROGRAM_CACHE = {}


def kernel(x, W_qkv, W_out):
    in_maps = prepare_in_maps(x, W_qkv, W_out)
    if 1 not in _PROGRAM_CACHE:
        _PROGRAM_CACHE[1] = build_program(1)
    nc = _PROGRAM_CACHE[1]
    res = run_bass_kernel_spmd(nc, in_maps, core_ids=list(range(N_CORES)))
    return combine_outputs(res.results)


# revision 14
# speedup vs baseline: 1.4101x; 1.0182x over previous
"""Causal self-attention on 8 Trainium2 NeuronCores.

Problem: x[2,2048,2048] f32, W_qkv[2048,6144], W_out[2048,2048]
  qkv = x @ W_qkv; per-head causal softmax attention; out = attn @ W_out.

Sharding: core c handles batch b=c//4, head group hg=c%4 (4 of 16 heads).
Each core computes its heads' QKV projections, full causal attention for
those heads, and a partial output projection (its heads' rows of W_out).
Host sums the 4 partial outputs per batch.

All device matmuls run in bf16 (fp32 PSUM accumulation): same PE rate as
fp32r at 512-wide outputs, but enables FWL fast weight loads and halves
DMA + SBUF. Host pre-packs every tensor so each DMA is a large
contiguous-per-partition transfer, and all weights are loaded into SBUF
exactly once (per-head q/k chunks so the first matmul group starts after
~2.5MB of DMA).

Per-slab interleave keeps the PE warm end to end: for each 512-query
slab s: [A] project q/k for the 4 local heads + v (weights resident,
xT slab double-buffered), then per head h: [B] causal attention for
query group s, then [C] one 128-row block of the output projection for
query group s-1 (attention outputs live in SBUF; the one-group lag
guarantees their normalize tails are done, and the C matmuls fill
TensorE bubbles left by the exp-latency chain).

Attention per (group, head): S^T = k_blk^T.T @ q^T with keys on
partitions; diagonal key-blocks are narrowed to their live query range
so S/exp/AV/denominator all skip fully-masked work, and only the first
128 columns of each diagonal block (the triangular part) get a
precomputed bf16 causal-mask multiply on VectorE. The softmax
denominator accumulates on TensorE as a [1,512] ones-matmul per
key-block (cheap: cost scales with streamed rows only), the reciprocal
uses the fast custom-DVE approximation, and a K=1 f32r ones-matmul
broadcasts it across partitions for the normalize multiply.
"""
import math

import ml_dtypes
import numpy as np

import concourse.bass as bass
import concourse.mybir as mybir
import concourse.tile as tile
from concourse import bacc
from concourse.bass_utils import run_bass_kernel_spmd

B, T, D = 2, 2048, 2048
H, Hd = 16, 128
N_CORES = 8
HL = 4            # heads per core
DL = HL * Hd      # 512: local hidden slice
P = 128
KC = D // P       # 16 contraction chunks of 128
NTB = T // P      # 16 row blocks of 128
QTW = 512         # query-group width
NQT = T // QTW    # 4 query groups
SCALE = 1.0 / math.sqrt(Hd)

f32 = mybir.dt.float32
f32r = mybir.dt.float32r
bf16 = mybir.dt.bfloat16
AF = mybir.ActivationFunctionType


def build_program(reps: int = 1, phases: str = "ABC"):
    nc = bacc.Bacc("TRN2", target_bir_lowering=False, debug=False,
                   num_devices=N_CORES)
    # host-packed layouts (partition dim first, contiguous per partition)
    xT = nc.dram_tensor("xT", [P, NQT, KC, QTW], bf16, kind="ExternalInput")
    wq = nc.dram_tensor("wq", [P, HL, KC, Hd], bf16, kind="ExternalInput")
    wk = nc.dram_tensor("wk", [P, HL, KC, Hd], bf16, kind="ExternalInput")
    wv = nc.dram_tensor("wv", [P, KC, DL], bf16, kind="ExternalInput")
    wout = nc.dram_tensor("wout", [P, HL, D], bf16, kind="ExternalInput")
    y = nc.dram_tensor("y", [T, D], bf16, kind="ExternalOutput")

    with tile.TileContext(nc) as tc:
        if reps > 1:
            with tc.For_i(0, reps, 1):
                _body(nc, tc, xT, wq, wk, wv, wout, y, phases)
        else:
            _body(nc, tc, xT, wq, wk, wv, wout, y, phases)
    nc.compile()
    return nc


def _body(nc, tc, xT, wq, wk, wv, wout, y, phases="ABC"):
    do_b = "B" in phases
    do_c = "C" in phases
    with (
        tc.tile_pool(name="weights", bufs=1) as wpool,
        tc.tile_pool(name="qkv", bufs=1) as qkv_pool,
        tc.tile_pool(name="consts", bufs=1) as cpool,
        tc.tile_pool(name="xts", bufs=2) as xpool,
        tc.tile_pool(name="e", bufs=6) as epool,
        tc.tile_pool(name="rec", bufs=2) as recpool,
        tc.tile_pool(name="oraw", bufs=2) as opool,
        tc.tile_pool(name="ysb", bufs=2) as ypool,
        tc.tile_pool(name="psum", bufs=1, space="PSUM") as psum,
    ):
        # causal mask / ones constants (f32 scratch -> target dtypes)
        ones_col = cpool.tile([P, 1], bf16)     # lhsT for denom matmul
        ones_row = cpool.tile([1, P], bf16)     # lhsT for K=1 broadcast
        tri = cpool.tile([P, P], bf16)          # tri[r, j] = 1.0 iff j >= r
        with tc.tile_pool(name="init_scratch", bufs=1) as scratch:
            sc = scratch.tile([P, P], f32)
            nc.gpsimd.memset(sc[:], 1.0)
            with nc.allow_low_precision(reason="0/1 mask exact in bf16"):
                nc.vector.tensor_copy(ones_col[:], sc[:, :1])
                nc.vector.tensor_copy(ones_row[:], sc[:1, :])
            nc.gpsimd.affine_select(
                out=sc[:], in_=sc[:],
                compare_op=mybir.AluOpType.is_ge,
                fill=0.0, base=0, channel_multiplier=-1,
                pattern=[[1, P]])
            with nc.allow_low_precision(reason="0/1 mask exact in bf16"):
                nc.vector.tensor_copy(tri[:], sc[:])

        wq_sb = wpool.tile([P, HL, KC, Hd], bf16)
        wk_sb = wpool.tile([P, HL, KC, Hd], bf16)
        wv_sb = wpool.tile([P, KC, DL], bf16)
        wout_sb = wpool.tile([P, HL, D], bf16)
        qT_sb = qkv_pool.tile([P, HL, T], bf16)   # [Hd, h, Tq]
        kT_sb = qkv_pool.tile([P, HL, T], bf16)
        v_sb = qkv_pool.tile([P, NTB, DL], bf16)  # [Tk%128, kb, h*Hd]
        atT_sb = qkv_pool.tile([P, HL, T], bf16)  # normalized attn outT

        def b_head_main(qt, h):
            o_ps = psum.tile([P, QTW], f32, tag="o", bufs=2, name="o_ps")
            d_ps = psum.tile([1, QTW], f32, tag="d", bufs=1, name="d_ps")
            nkb = (qt + 1) * 4
            for kb in range(nkb):
                m = kb - 4 * qt  # >=0: diagonal block, narrow to live cols
                off = max(m, 0) * P
                q_sl = slice(qt * QTW + off, (qt + 1) * QTW)
                s_ps = psum.tile([P, QTW], f32, tag="s", bufs=3, name="s_ps")
                nc.tensor.matmul(
                    s_ps[:, off:],
                    kT_sb[:, h, kb * P:(kb + 1) * P],
                    qT_sb[:, h, q_sl],
                    start=True, stop=True)
                e_sb = epool.tile([P, QTW], bf16, tag="e")
                with nc.allow_low_precision(reason="bf16 attn weights"):
                    nc.scalar.activation(
                        e_sb[:, off:], s_ps[:, off:], AF.Exp,
                        scale=float(SCALE))
                if m >= 0:
                    # only the first 128 live columns are partially masked
                    nc.vector.tensor_mul(
                        e_sb[:, off:off + P], e_sb[:, off:off + P], tri[:])
                nc.tensor.matmul(
                    o_ps[:, off:],
                    v_sb[:, kb, h * Hd:(h + 1) * Hd],
                    e_sb[:, off:],
                    start=(kb == 0), stop=(kb == nkb - 1))
                nc.tensor.matmul(
                    d_ps[:, off:], ones_col[:], e_sb[:, off:],
                    start=(kb == 0), stop=(kb == nkb - 1))
            return o_ps, d_ps

        # the reciprocal -> broadcast chain takes ~1us after the last
        # denominator matmul; emitting it after the next block of
        # independent matmuls keeps TensorE's FIFO from stalling on it
        def b_head_tail(qt, h, o_ps, d_ps):
            rec = recpool.tile([1, QTW], f32, tag="rec")
            nc.vector.reciprocal_approx_fast(rec[:], d_ps[:])
            rec_bf = recpool.tile([1, QTW], bf16, tag="recbf")
            with nc.allow_low_precision(reason="bf16 softmax recip"):
                nc.vector.tensor_copy(rec_bf[:], rec[:])
            bc_ps = psum.tile([P, QTW], f32, tag="s", bufs=3, name="bc_ps")
            nc.tensor.matmul(
                bc_ps[:], ones_row[:], rec_bf[:],
                start=True, stop=True)
            o_raw = opool.tile([P, QTW], bf16, tag="oraw")
            with nc.allow_low_precision(reason="bf16 attn out"):
                nc.vector.tensor_copy(o_raw[:], o_ps[:])
                nc.vector.tensor_mul(
                    atT_sb[:, h, qt * QTW:(qt + 1) * QTW], o_raw[:],
                    bc_ps[:])

        def c_tb(tb):
            t_sl = slice(tb * P, (tb + 1) * P)
            y_sb = ypool.tile([P, D], bf16, tag="ysb")
            for dc in range(D // QTW):
                y_ps = psum.tile([P, QTW], f32, tag="mm", bufs=2, name="y_ps")
                for h in range(HL):
                    nc.tensor.matmul(
                        y_ps[:],
                        atT_sb[:, h, t_sl],
                        wout_sb[:, h, dc * QTW:(dc + 1) * QTW],
                        start=(h == 0), stop=(h == HL - 1))
                with nc.allow_low_precision(reason="bf16 partial y"):
                    nc.vector.tensor_copy(y_sb[:, dc * QTW:(dc + 1) * QTW],
                                          y_ps[:])
            nc.sync.dma_start(y.ap()[t_sl, :], y_sb[:])

        # first xT slab first (split in 4 so it spreads across DMA
        # queues) so the first matmul group isn't queued behind the
        # full weight load; wout (not needed until the first C block,
        # one slab later) goes last
        xTs0 = xpool.tile([P, KC, QTW], bf16, tag="xT")
        for c4 in range(4):
            nc.sync.dma_start(xTs0[:, 4 * c4:4 * (c4 + 1)],
                              xT.ap()[:, 0, 4 * c4:4 * (c4 + 1)])
        for h in range(HL):
            nc.sync.dma_start(wq_sb[:, h], wq.ap()[:, h])
            nc.sync.dma_start(wk_sb[:, h], wk.ap()[:, h])
        nc.sync.dma_start(wv_sb[:], wv.ap())
        if do_c:
            nc.sync.dma_start(wout_sb[:], wout.ap())

        for s in range(NQT):
            # ---- A: qkv projection for this 512-query slab --------------
            if s == 0:
                xTs = xTs0
            else:
                xTs = xpool.tile([P, KC, QTW], bf16, tag="xT")
                nc.sync.dma_start(xTs[:], xT.ap()[:, s])
            for h in range(HL):
                for w_sb, dst in ((wq_sb, qT_sb), (wk_sb, kT_sb)):
                    ps = psum.tile([P, QTW], f32, tag="mm", bufs=2,
                                   name="qk_ps")
                    for kc in range(KC):
                        nc.tensor.matmul(
                            ps[:], w_sb[:, h, kc], xTs[:, kc, :],
                            start=(kc == 0), stop=(kc == KC - 1))
                    with nc.allow_low_precision(reason="bf16 qkv"):
                        nc.vector.tensor_copy(
                            dst[:, h, s * QTW:(s + 1) * QTW], ps[:])
            for tsub in range(4):
                ps = psum.tile([P, DL], f32, tag="mm", bufs=2, name="v_ps")
                for kc in range(KC):
                    nc.tensor.matmul(
                        ps[:], xTs[:, kc, tsub * P:(tsub + 1) * P],
                        wv_sb[:, kc, :],
                        start=(kc == 0), stop=(kc == KC - 1))
                with nc.allow_low_precision(reason="bf16 qkv"):
                    nc.vector.tensor_copy(v_sb[:, s * 4 + tsub, :], ps[:])
            # ---- B qt=s, interleaved with C for query group s-1 ---------
            if not do_b:
                continue
            for h in range(HL):
                handle = b_head_main(s, h)
                if do_c and s > 0:
                    c_tb(4 * (s - 1) + h)
                b_head_tail(s, h, *handle)
        if do_b and do_c:
            for tb in range(4 * (NQT - 1), 4 * NQT):
                c_tb(tb)


def prepare_in_maps(x, W_qkv, W_out):
    x = np.ascontiguousarray(np.asarray(x), dtype=np.float32)
    W_qkv = np.ascontiguousarray(np.asarray(W_qkv), dtype=np.float32)
    W_out = np.ascontiguousarray(np.asarray(W_out), dtype=np.float32)
    bf = ml_dtypes.bfloat16
    Wr = W_qkv.reshape(D, 3, H, Hd)
    Wo = W_out.reshape(H, Hd, D)
    # xT[b]: [D,T] -> [p, s, kc, t]
    xTs = [np.ascontiguousarray(
        x[b].T.reshape(KC, P, NQT, QTW).transpose(1, 2, 0, 3).astype(bf))
        for b in range(B)]

    def packw_h(w):  # [D, DL] -> [p, h, kc, hd]
        return np.ascontiguousarray(
            w.reshape(KC, P, HL, Hd).transpose(1, 2, 0, 3).astype(bf))

    def packw(w):  # [D, DL] -> [p, kc, DL]
        return np.ascontiguousarray(
            w.reshape(KC, P, DL).transpose(1, 0, 2).astype(bf))

    in_maps = []
    for c in range(N_CORES):
        b, hg = c // 4, c % 4
        hs = slice(hg * HL, (hg + 1) * HL)
        in_maps.append({
            "xT": xTs[b],
            "wq": packw_h(Wr[:, 0, hs, :].reshape(D, DL)),
            "wk": packw_h(Wr[:, 1, hs, :].reshape(D, DL)),
            "wv": packw(Wr[:, 2, hs, :].reshape(D, DL)),
            "wout": np.ascontiguousarray(
                Wo[hs].transpose(1, 0, 2).astype(bf)),
        })
    return in_maps


def combine_outputs(results):
    out = np.zeros((B, T, D), dtype=np.float32)
    for c in range(N_CORES):
        out[c // 4] += results[c]["y"].astype(np.float32)
    return out


_PROGRAM_CACHE = {}


def kernel(x, W_qkv, W_out):
    in_maps = prepare_in_maps(x, W_qkv, W_out)
    if 1 not in _PROGRAM_CACHE:
        _PROGRAM_CACHE[1] = build_program(1)
    nc = _PROGRAM_CACHE[1]
    res = run_bass_kernel_spmd(nc, in_maps, core_ids=list(range(N_CORES)))
    return combine_outputs(res.results)


# revision 16
# speedup vs baseline: 1.5036x; 1.0664x over previous
"""Causal self-attention on 8 Trainium2 NeuronCores.

Problem: x[2,2048,2048] f32, W_qkv[2048,6144], W_out[2048,2048]
  qkv = x @ W_qkv; per-head causal softmax attention; out = attn @ W_out.

Sharding: core c handles batch b=c//4, head group hg=c%4 (4 of 16 heads).
Each core computes its heads' QKV projections, full causal attention for
those heads, and a partial output projection (its heads' rows of W_out).
Host sums the 4 partial outputs per batch.

All device matmuls run in bf16 (fp32 PSUM accumulation): same PE rate as
fp32r at 512-wide outputs, but enables FWL fast weight loads and halves
DMA + SBUF. Host pre-packs every tensor so each DMA is a large
contiguous-per-partition transfer, and all weights are loaded into SBUF
exactly once. Partial y outputs leave in bf16 (summed in f32 on host).

Structure: QKV projection is split into per-512-query slabs; attention
for query group s and its out-projection run as one interleaved stream.
Each attention head emits: scores S^T = k_blk^T.T @ q^T (keys on
partitions; diagonal key-blocks narrowed to their live query range,
with only the triangular first 128 columns getting a precomputed bf16
mask multiply), exp on ScalarE, AV accumulation, and softmax-denominator
accumulation on VectorE (even blocks) + GpSimd (odd blocks) into two
f32r accumulators so TensorE does no per-block reduction work. The
denominator is then closed with two f32r ones-matmuls, a fast custom-DVE
reciprocal, and a K=1 broadcast matmul feeding the normalize multiply.

Between those latency-chained steps the emitter inserts independent
"filler" work — out-projection 128-row blocks for the previous query
group and next-slab QKV projection groups — so the TensorE FIFO never
stalls on the exp/reciprocal chains. C blocks are deliberately skewed
toward the last segment, whose exp load is largest. This keeps the PE
busy (and HAM un-throttled) end to end.
"""
import math

import ml_dtypes
import numpy as np

import concourse.bass as bass
import concourse.mybir as mybir
import concourse.tile as tile
from concourse import bacc
from concourse.bass_utils import run_bass_kernel_spmd

B, T, D = 2, 2048, 2048
H, Hd = 16, 128
N_CORES = 8
HL = 4            # heads per core
DL = HL * Hd      # 512: local hidden slice
P = 128
KC = D // P       # 16 contraction chunks of 128
NTB = T // P      # 16 row blocks of 128
QTW = 512         # query-group width
NQT = T // QTW    # 4 query groups
SCALE = 1.0 / math.sqrt(Hd)

f32 = mybir.dt.float32
f32r = mybir.dt.float32r
bf16 = mybir.dt.bfloat16
AF = mybir.ActivationFunctionType

# out-projection row-blocks emitted inside each segment (qt groups of
# the blocks must already be finished): skewed late because the last
# segments have the most exp work to hide
C_ASSIGN = {1: [0, 1], 2: [2, 3, 4, 5], 3: [6, 7, 8, 9, 10, 11]}
C_FINAL = [12, 13, 14, 15]


def build_program(reps: int = 1, phases: str = "ABC"):
    nc = bacc.Bacc("TRN2", target_bir_lowering=False, debug=False,
                   num_devices=N_CORES)
    # host-packed layouts (partition dim first, contiguous per partition)
    xT = nc.dram_tensor("xT", [P, NQT, KC, QTW], bf16, kind="ExternalInput")
    wq = nc.dram_tensor("wq", [P, HL, KC, Hd], bf16, kind="ExternalInput")
    wk = nc.dram_tensor("wk", [P, HL, KC, Hd], bf16, kind="ExternalInput")
    wv = nc.dram_tensor("wv", [P, KC, DL], bf16, kind="ExternalInput")
    wout = nc.dram_tensor("wout", [P, HL, D], bf16, kind="ExternalInput")
    y = nc.dram_tensor("y", [T, D], bf16, kind="ExternalOutput")

    with tile.TileContext(nc) as tc:
        if reps > 1:
            with tc.For_i(0, reps, 1):
                _body(nc, tc, xT, wq, wk, wv, wout, y, phases)
        else:
            _body(nc, tc, xT, wq, wk, wv, wout, y, phases)
    nc.compile()
    return nc


def _body(nc, tc, xT, wq, wk, wv, wout, y, phases="ABC"):
    do_b = "B" in phases
    do_c = "C" in phases
    with (
        tc.tile_pool(name="weights", bufs=1) as wpool,
        tc.tile_pool(name="qkv", bufs=1) as qkv_pool,
        tc.tile_pool(name="consts", bufs=1) as cpool,
        tc.tile_pool(name="xts", bufs=2) as xpool,
        tc.tile_pool(name="e", bufs=6) as epool,
        tc.tile_pool(name="eacc", bufs=2) as eaccpool,
        tc.tile_pool(name="rec", bufs=2) as recpool,
        tc.tile_pool(name="oraw", bufs=2) as opool,
        tc.tile_pool(name="ysb", bufs=2) as ypool,
        tc.tile_pool(name="psum", bufs=1, space="PSUM") as psum,
    ):
        # causal mask / ones constants (f32 scratch -> target dtypes)
        ones_col = cpool.tile([P, 1], f32r)     # lhsT for denom matmuls
        ones_row = cpool.tile([1, P], bf16)     # lhsT for K=1 broadcast
        tri = cpool.tile([P, P], bf16)          # tri[r, j] = 1.0 iff j >= r
        with tc.tile_pool(name="init_scratch", bufs=1) as scratch:
            sc = scratch.tile([P, P], f32)
            nc.gpsimd.memset(sc[:], 1.0)
            with nc.allow_low_precision(reason="exact small constants"):
                nc.vector.tensor_copy(ones_col[:], sc[:, :1])
                nc.vector.tensor_copy(ones_row[:], sc[:1, :])
            nc.gpsimd.affine_select(
                out=sc[:], in_=sc[:],
                compare_op=mybir.AluOpType.is_ge,
                fill=0.0, base=0, channel_multiplier=-1,
                pattern=[[1, P]])
            with nc.allow_low_precision(reason="0/1 mask exact in bf16"):
                nc.vector.tensor_copy(tri[:], sc[:])

        wq_sb = wpool.tile([P, HL, KC, Hd], bf16)
        wk_sb = wpool.tile([P, HL, KC, Hd], bf16)
        wv_sb = wpool.tile([P, KC, DL], bf16)
        wout_sb = wpool.tile([P, HL, D], bf16)
        qT_sb = qkv_pool.tile([P, HL, T], bf16)   # [Hd, h, Tq]
        kT_sb = qkv_pool.tile([P, HL, T], bf16)
        v_sb = qkv_pool.tile([P, NTB, DL], bf16)  # [Tk%128, kb, h*Hd]
        atT_sb = qkv_pool.tile([P, HL, T], bf16)  # normalized attn outT

        def a_slab_dma(s):
            xTs = xpool.tile([P, KC, QTW], bf16, tag="xT", name="xTs")
            for c4 in range(4):
                nc.sync.dma_start(xTs[:, 4 * c4:4 * (c4 + 1)],
                                  xT.ap()[:, s, 4 * c4:4 * (c4 + 1)])
            return xTs

        def a_qk_group(xTs, s, h, w_sb, dst):
            ps = psum.tile([P, QTW], f32, tag="mm", bufs=2, name="qk_ps")
            for kc in range(KC):
                nc.tensor.matmul(
                    ps[:], w_sb[:, h, kc], xTs[:, kc, :],
                    start=(kc == 0), stop=(kc == KC - 1))
            with nc.allow_low_precision(reason="bf16 qkv"):
                nc.vector.tensor_copy(
                    dst[:, h, s * QTW:(s + 1) * QTW], ps[:])

        def a_v_group(xTs, s, tsub):
            ps = psum.tile([P, DL], f32, tag="mm", bufs=2, name="v_ps")
            for kc in range(KC):
                nc.tensor.matmul(
                    ps[:], xTs[:, kc, tsub * P:(tsub + 1) * P],
                    wv_sb[:, kc, :],
                    start=(kc == 0), stop=(kc == KC - 1))
            with nc.allow_low_precision(reason="bf16 qkv"):
                nc.vector.tensor_copy(v_sb[:, s * 4 + tsub, :], ps[:])

        def a_units(xTs, s):
            units = []
            for h in range(HL):
                units.append(lambda h=h: a_qk_group(xTs, s, h, wq_sb, qT_sb))
                units.append(lambda h=h: a_qk_group(xTs, s, h, wk_sb, kT_sb))
            for tsub in range(4):
                units.append(lambda t=tsub: a_v_group(xTs, s, t))
            return units

        def b_head_main(qt, h):
            o_ps = psum.tile([P, QTW], f32, tag="o", bufs=2, name="o_ps")
            ea_v = eaccpool.tile([P, QTW], f32r, tag="eav", name="ea_v")
            ea_g = eaccpool.tile([P, QTW], f32r, tag="eag", name="ea_g")
            nkb = (qt + 1) * 4
            for kb in range(nkb):
                m = kb - 4 * qt  # >=0: diagonal block, narrow to live cols
                off = max(m, 0) * P
                q_sl = slice(qt * QTW + off, (qt + 1) * QTW)
                s_ps = psum.tile([P, QTW], f32, tag="s", bufs=3, name="s_ps")
                nc.tensor.matmul(
                    s_ps[:, off:],
                    kT_sb[:, h, kb * P:(kb + 1) * P],
                    qT_sb[:, h, q_sl],
                    start=True, stop=True)
                e_sb = epool.tile([P, QTW], bf16, tag="e")
                with nc.allow_low_precision(reason="bf16 attn weights"):
                    nc.scalar.activation(
                        e_sb[:, off:], s_ps[:, off:], AF.Exp,
                        scale=float(SCALE))
                if m >= 0:
                    # only the first 128 live columns are partially masked
                    nc.vector.tensor_mul(
                        e_sb[:, off:off + P], e_sb[:, off:off + P], tri[:])
                nc.tensor.matmul(
                    o_ps[:, off:],
                    v_sb[:, kb, h * Hd:(h + 1) * Hd],
                    e_sb[:, off:],
                    start=(kb == 0), stop=(kb == nkb - 1))
                # denominator accumulate, alternating engines so neither
                # becomes the segment bottleneck
                eng, acc = (nc.vector, ea_v) if kb % 2 == 0 else \
                    (nc.gpsimd, ea_g)
                with nc.allow_low_precision(reason="f32r denom accum"):
                    if kb < 2:
                        eng.tensor_copy(acc[:, off:], e_sb[:, off:])
                        if off:
                            eng.memset(
                                acc[:, :off].bitcast(mybir.dt.uint32), 0)
                    else:
                        eng.tensor_add(
                            acc[:, off:], acc[:, off:], e_sb[:, off:])
            return o_ps, ea_v, ea_g

        def b_head_denom(ea_v, ea_g):
            d_ps = psum.tile([1, QTW], f32, tag="d", bufs=1, name="d_ps")
            nc.tensor.matmul(d_ps[:], ones_col[:], ea_v[:],
                             start=True, stop=False)
            nc.tensor.matmul(d_ps[:], ones_col[:], ea_g[:],
                             start=False, stop=True)
            return d_ps

        def b_head_fin(qt, h, o_ps, d_ps):
            rec = recpool.tile([1, QTW], f32, tag="rec")
            nc.vector.reciprocal_approx_fast(rec[:], d_ps[:])
            rec_bf = recpool.tile([1, QTW], bf16, tag="recbf")
            with nc.allow_low_precision(reason="bf16 softmax recip"):
                nc.vector.tensor_copy(rec_bf[:], rec[:])
            bc_ps = psum.tile([P, QTW], f32, tag="s", bufs=3, name="bc_ps")
            nc.tensor.matmul(
                bc_ps[:], ones_row[:], rec_bf[:],
                start=True, stop=True)
            o_raw = opool.tile([P, QTW], bf16, tag="oraw")
            with nc.allow_low_precision(reason="bf16 attn out"):
                nc.vector.tensor_copy(o_raw[:], o_ps[:])
                nc.vector.tensor_mul(
                    atT_sb[:, h, qt * QTW:(qt + 1) * QTW], o_raw[:],
                    bc_ps[:])

        def c_units(tb):
            t_sl = slice(tb * P, (tb + 1) * P)
            y_sb = ypool.tile([P, D], bf16, tag="ysb", name="y_sb")

            def unit(dc):
                y_ps = psum.tile([P, QTW], f32, tag="mm", bufs=2,
                                 name="y_ps")
                for h in range(HL):
                    nc.tensor.matmul(
                        y_ps[:],
                        atT_sb[:, h, t_sl],
                        wout_sb[:, h, dc * QTW:(dc + 1) * QTW],
                        start=(h == 0), stop=(h == HL - 1))
                with nc.allow_low_precision(reason="bf16 partial y"):
                    nc.vector.tensor_copy(
                        y_sb[:, dc * QTW:(dc + 1) * QTW], y_ps[:])
                if dc == 1:
                    nc.sync.dma_start(y.ap()[t_sl, :D // 2],
                                      y_sb[:, :D // 2])
                elif dc == 3:
                    nc.sync.dma_start(y.ap()[t_sl, D // 2:],
                                      y_sb[:, D // 2:])

            return [lambda dc=dc: unit(dc) for dc in range(4)]

        # ---- initial DMAs: first xT chunks interleaved with the first
        # head's q/k weights so the first matmul group starts early
        xTs0 = xpool.tile([P, KC, QTW], bf16, tag="xT", name="xTs")
        nc.sync.dma_start(xTs0[:, 0:4], xT.ap()[:, 0, 0:4])
        nc.sync.dma_start(wq_sb[:, 0], wq.ap()[:, 0])
        nc.sync.dma_start(xTs0[:, 4:8], xT.ap()[:, 0, 4:8])
        nc.sync.dma_start(wk_sb[:, 0], wk.ap()[:, 0])
        nc.sync.dma_start(xTs0[:, 8:12], xT.ap()[:, 0, 8:12])
        nc.sync.dma_start(xTs0[:, 12:16], xT.ap()[:, 0, 12:16])
        for h in range(1, HL):
            nc.sync.dma_start(wq_sb[:, h], wq.ap()[:, h])
            nc.sync.dma_start(wk_sb[:, h], wk.ap()[:, h])
        nc.sync.dma_start(wv_sb[:], wv.ap())
        if do_c:
            nc.sync.dma_start(wout_sb[:], wout.ap())

        # ---- slab 0 projection up front (nothing to interleave with)
        for u in a_units(xTs0, 0):
            u()
        if not do_b:
            for s in range(1, NQT):
                for u in a_units(a_slab_dma(s), s):
                    u()
            return
        xTs_next = a_slab_dma(1)

        # ---- segments: B(s) heads + filler units ------------------------
        for s in range(NQT):
            units = []
            if do_c and s in C_ASSIGN:
                for tb in C_ASSIGN[s]:
                    units.extend(c_units(tb))
            if s < NQT - 1:
                units.extend(a_units(xTs_next, s + 1))
            # distribute across the 4 heads: ~1/4 each, >=2 after denom
            per = [units[(len(units) * h) // HL:
                         (len(units) * (h + 1)) // HL] for h in range(HL)]
            for h in range(HL):
                handle = b_head_main(s, h)
                mine = per[h]
                if mine:
                    mine[0]()
                d_ps = b_head_denom(handle[1], handle[2])
                for u in mine[1:]:
                    u()
                b_head_fin(s, h, handle[0], d_ps)
                if h == HL - 1 and s < NQT - 2:
                    xTs_next = a_slab_dma(s + 2)
        if do_c:
            for tb in C_FINAL:
                for u in c_units(tb):
                    u()


def prepare_in_maps(x, W_qkv, W_out):
    x = np.ascontiguousarray(np.asarray(x), dtype=np.float32)
    W_qkv = np.ascontiguousarray(np.asarray(W_qkv), dtype=np.float32)
    W_out = np.ascontiguousarray(np.asarray(W_out), dtype=np.float32)
    bf = ml_dtypes.bfloat16
    Wr = W_qkv.reshape(D, 3, H, Hd)
    Wo = W_out.reshape(H, Hd, D)
    # xT[b]: [D,T] -> [p, s, kc, t]
    xTs = [np.ascontiguousarray(
        x[b].T.reshape(KC, P, NQT, QTW).transpose(1, 2, 0, 3).astype(bf))
        for b in range(B)]

    def packw_h(w):  # [D, DL] -> [p, h, kc, hd]
        return np.ascontiguousarray(
            w.reshape(KC, P, HL, Hd).transpose(1, 2, 0, 3).astype(bf))

    def packw(w):  # [D, DL] -> [p, kc, DL]
        return np.ascontiguousarray(
            w.reshape(KC, P, DL).transpose(1, 0, 2).astype(bf))

    in_maps = []
    for c in range(N_CORES):
        b, hg = c // 4, c % 4
        hs = slice(hg * HL, (hg + 1) * HL)
        in_maps.append({
            "xT": xTs[b],
            "wq": packw_h(Wr[:, 0, hs, :].reshape(D, DL)),
            "wk": packw_h(Wr[:, 1, hs, :].reshape(D, DL)),
            "wv": packw(Wr[:, 2, hs, :].reshape(D, DL)),
            "wout": np.ascontiguousarray(
                Wo[hs].transpose(1, 0, 2).astype(bf)),
        })
    return in_maps


def combine_outputs(results):
    out = np.zeros((B, T, D), dtype=np.float32)
    for c in range(N_CORES):
        out[c // 4] += results[c]["y"].astype(np.float32)
    return out


_PROGRAM_CACHE = {}


def kernel(x, W_qkv, W_out):
    in_maps = prepare_in_maps(x, W_qkv, W_out)
    if 1 not in _PROGRAM_CACHE:
        _PROGRAM_CACHE[1] = build_program(1)
    nc = _PROGRAM_CACHE[1]
    res = run_bass_kernel_spmd(nc, in_maps, core_ids=list(range(N_CORES)))
    return combine_outputs(res.results)
